# revision 5
# baseline (speedup 1.0000x reference)
"""Trainium2 Bass kernel for nn_CRF_79551384256937 (CRF negative-log-likelihood loss).

Strategy (data-parallel over batch, 16 sequences per core, 8 cores):
  Forward partition function as a *multiplicative* scan in [tag, batch] layout:
      P_{t+1} = (expM^T @ P_t) * exp(u_t - c*),   expM[k, j] = exp(trans[j, k])
  with c* = log(254) + 0.5 a constant stabilizer (keeps P bounded in fp32/bf16,
  no renormalization needed).  Per step: 4 [128,128]x[128,16] bf16 matmuls
  (PSUM f32 accumulate) + DVE multiply.  r_raw[t] = exp(trans[end,:]) . P_{t+1}
  accumulated into PSUM columns (32 steps per bank), logged in bulk at the end;
  fwd[b] = log(r_raw[len_b - 1, b]) + len_b * c*.
  Gold score: emissions via host-built one-hot mask O (elementwise mul + reduce
  of the same transposed-u tiles), transitions via host-built pair-count
  histogram CNT contracted with trans on the tensor engine.
All tag/length-derived index structures (one-hots, counts, masks) are prepared
on host; every floating-point reduction over model data runs on device.
"""
import numpy as np
import ml_dtypes
from contextlib import ExitStack

import concourse.bass as bass
import concourse.bacc as bacc
import concourse.tile as tile
from concourse import mybir
from concourse.bass import MemorySpace
from concourse.bass_utils import run_bass_kernel_spmd

BF = ml_dtypes.bfloat16
F32 = np.float32

N_CORES = 8
B, T, NT = 128, 1024, 254
N = NT + 2            # 256 tags incl <GO>/<EOS>
BL = B // N_CORES     # 16 sequences per core
TC = 128              # time steps per chunk
NCH = T // TC         # 8 chunks
NEG = -10000.0
CSTAR = float(np.log(254.0) + 0.5)
GRP = 32              # r-row steps per PSUM bank
NGRP = T // GRP       # 32 groups

_compiled = {}


def _build_nc():
    nc = bacc.Bacc("TRN2", target_bir_lowering=False, debug=False,
                   num_devices=N_CORES)
    dt = mybir.dt
    # ---- DRAM I/O (per-core shapes) ----
    u_pad = nc.dram_tensor("u_pad", [T * BL, N], dt.bfloat16, kind="ExternalInput").ap()
    O_in = nc.dram_tensor("onehot", [N, T * BL], dt.bfloat16, kind="ExternalInput").ap()
    cnt_in = nc.dram_tensor("cnt", [128, 512 * BL], dt.float32, kind="ExternalInput").ap()
    transT_in = nc.dram_tensor("transT", [N, N], dt.float32, kind="ExternalInput").ap()
    tg_in = nc.dram_tensor("trans_gold", [128, 512], dt.float32, kind="ExternalInput").ap()
    p0_in = nc.dram_tensor("p0", [N, BL], dt.bfloat16, kind="ExternalInput").ap()
    msel_in = nc.dram_tensor("msel", [NGRP, GRP * BL], dt.float32, kind="ExternalInput").ap()
    lenc_in = nc.dram_tensor("lenc", [1, BL], dt.float32, kind="ExternalInput").ap()
    ones_in = nc.dram_tensor("ones", [128, 1], dt.float32, kind="ExternalInput").ap()
    out_d = nc.dram_tensor("out", [1, BL], dt.float32, kind="ExternalOutput").ap()

    with tile.TileContext(nc) as tc:
        with ExitStack() as ctx:
            singles = ctx.enter_context(tc.tile_pool(name="singles", bufs=1))
            chunks = ctx.enter_context(tc.tile_pool(name="chunks", bufs=2))
            ppool = ctx.enter_context(tc.tile_pool(name="ppool", bufs=3))
            spsum = ctx.enter_context(
                tc.tile_pool(name="spsum", bufs=2, space=MemorySpace.PSUM))
            rpsum = ctx.enter_context(
                tc.tile_pool(name="rpsum", bufs=2, space=MemorySpace.PSUM))
            gpsum = ctx.enter_context(
                tc.tile_pool(name="gpsum", bufs=1, space=MemorySpace.PSUM))

            # ---- constants / singles ----
            tT = [singles.tile([128, N], dt.float32, name=f"tT{h}") for h in (0, 1)]
            expM = [singles.tile([128, N], dt.bfloat16, name=f"expM{h}") for h in (0, 1)]
            for h in (0, 1):
                nc.sync.dma_start(out=tT[h], in_=transT_in[128 * h:128 * (h + 1), :])
                nc.scalar.activation(out=expM[h], in_=tT[h],
                                     func=mybir.ActivationFunctionType.Exp)
            Pinit = singles.tile([128, 2 * BL], dt.bfloat16)
            for h in (0, 1):
                nc.sync.dma_start(out=Pinit[:, BL * h:BL * (h + 1)],
                                  in_=p0_in[128 * h:128 * (h + 1), :])
            cnt_sb = singles.tile([128, 512 * BL], dt.float32)
            nc.sync.dma_start(out=cnt_sb, in_=cnt_in)
            tg_sb = singles.tile([128, 512], dt.float32)
            nc.sync.dma_start(out=tg_sb, in_=tg_in)
            ones_sb = singles.tile([128, 1], dt.float32)
            nc.sync.dma_start(out=ones_sb, in_=ones_in)
            msel_sb = singles.tile([NGRP, GRP * BL], dt.float32)
            nc.sync.dma_start(out=msel_sb, in_=msel_in)
            lenc_sb = singles.tile([1, BL], dt.float32)
            nc.sync.dma_start(out=lenc_sb, in_=lenc_in)
            gacc = singles.tile([128, BL], dt.float32)
            cbias = singles.tile([128, 1], dt.float32)
            nc.vector.memset(cbias, -CSTAR)
            nc.vector.memset(gacc, 0.0)
            rbuf = singles.tile([NGRP, GRP * BL], dt.float32)

            # ---- gold transition score: 512 accumulating [128,1]x[128,16] matmuls ----
            gt_ps = gpsum.tile([1, BL], dt.float32, tag="gt")
            for c in range(512):
                nc.tensor.matmul(gt_ps, tg_sb[:, c:c + 1],
                                 cnt_sb[:, BL * c:BL * (c + 1)],
                                 start=(c == 0), stop=(c == 511))

            # ---- the scan ----
            Pprev = Pinit
            pend_r = None  # deferred r-hat matmul args: (Pn, t)
            rp = None

            def emit_r(Pn, t):
                nonlocal rp
                g, s = t // GRP, t % GRP
                if s == 0:
                    rp = rpsum.tile([1, GRP * BL], dt.float32, tag="rp")
                for h in (0, 1):
                    nc.tensor.matmul(rp[:, BL * s:BL * (s + 1)],
                                     expM[h][:, N - 1:N],
                                     Pn[:, BL * h:BL * (h + 1)],
                                     start=(h == 0), stop=(h == 1))
                if s == GRP - 1:
                    stg = ppool.tile([1, GRP * BL], dt.float32, tag="rstg")
                    nc.scalar.copy(out=stg, in_=rp)
                    nc.sync.dma_start(out=rbuf[g:g + 1, :], in_=stg)

            for ch in range(NCH):
                uT = chunks.tile([128, 2 * TC * BL], dt.bfloat16, tag="uT")
                for h in (0, 1):
                    nc.sync.dma_start_transpose(
                        out=uT[:, TC * BL * h:TC * BL * (h + 1)],
                        in_=u_pad[ch * TC * BL:(ch + 1) * TC * BL,
                                  128 * h:128 * (h + 1)])
                eU = chunks.tile([128, 2 * TC * BL], dt.bfloat16, tag="eU")
                nc.scalar.activation(out=eU, in_=uT,
                                     func=mybir.ActivationFunctionType.Exp,
                                     bias=cbias[:, :])
                # gold emission pieces for this chunk
                Ot = chunks.tile([128, 2 * TC * BL], dt.bfloat16, tag="Ot")
                for h in (0, 1):
                    nc.sync.dma_start(
                        out=Ot[:, TC * BL * h:TC * BL * (h + 1)],
                        in_=O_in[128 * h:128 * (h + 1),
                                 ch * TC * BL:(ch + 1) * TC * BL])
                gp = chunks.tile([128, 2 * TC * BL], dt.bfloat16, tag="gp")
                for h in (0, 1):
                    sl = slice(TC * BL * h, TC * BL * (h + 1))
                    nc.gpsimd.tensor_mul(gp[:, sl], Ot[:, sl], uT[:, sl])
                for piece in range(8):  # 8 pieces of [128, 512] -> [128, 16]
                    src = gp[:, 512 * piece:512 * (piece + 1)].rearrange(
                        "p (s b) -> p b s", b=BL)
                    rtmp = ppool.tile([128, BL], dt.float32, tag="rtmp")
                    nc.vector.tensor_reduce(rtmp, src, axis=mybir.AxisListType.X,
                                            op=mybir.AluOpType.add)
                    nc.vector.tensor_add(gacc, gacc, rtmp)

                for s in range(TC):
                    t = ch * TC + s
                    S = spsum.tile([128, 2 * BL], dt.float32, tag="S")
                    for g in (0, 1):
                        for h in (0, 1):
                            nc.tensor.matmul(
                                S[:, BL * g:BL * (g + 1)],
                                expM[h][:, 128 * g:128 * (g + 1)],
                                Pprev[:, BL * h:BL * (h + 1)],
                                start=(h == 0), stop=(h == 1))
                    if pend_r is not None:
                        emit_r(*pend_r)
                    Pn = ppool.tile([128, 2 * BL], dt.bfloat16, tag="P")
                    for h in (0, 1):
                        nc.vector.tensor_mul(
                            Pn[:, BL * h:BL * (h + 1)],
                            S[:, BL * h:BL * (h + 1)],
                            eU[:, TC * BL * h + BL * s:TC * BL * h + BL * (s + 1)])
                    pend_r = (Pn, t)
                    Pprev = Pn
            emit_r(*pend_r)

            # ---- final assembly ----
            rlog = singles.tile([NGRP, GRP * BL], dt.float32)
            nc.scalar.activation(out=rlog, in_=rbuf,
                                 func=mybir.ActivationFunctionType.Ln)
            rm = singles.tile([NGRP, GRP * BL], dt.float32)
            nc.vector.tensor_mul(rm, rlog, msel_sb)
            rsum = singles.tile([NGRP, BL], dt.float32)
            nc.vector.tensor_reduce(
                rsum, rm.rearrange("p (s b) -> p b s", b=BL),
                axis=mybir.AxisListType.X, op=mybir.AluOpType.add)
            rsel_ps = gpsum.tile([1, BL], dt.float32, tag="rsel")
            nc.tensor.matmul(rsel_ps, ones_sb[0:NGRP, :], rsum, start=True, stop=True)
            ge_ps = gpsum.tile([1, BL], dt.float32, tag="ge")
            nc.tensor.matmul(ge_ps, ones_sb, gacc, start=True, stop=True)

            x1 = singles.tile([1, BL], dt.float32, tag="x1")
            nc.vector.tensor_add(x1, rsel_ps, lenc_sb)
            x2 = singles.tile([1, BL], dt.float32, tag="x2")
            nc.vector.tensor_sub(x2, x1, ge_ps)
            x3 = singles.tile([1, BL], dt.float32, tag="x3")
            nc.vector.tensor_sub(x3, x2, gt_ps)
            nc.sync.dma_start(out=out_d, in_=x3)

    nc.compile()
    return nc


def _host_prep(unary, tags, lengths, transitions):
    """Build the 8 per-core input maps (index prep + layout only)."""
    unary = np.asarray(unary, dtype=F32)
    tags = np.asarray(tags).astype(np.int64)
    lengths = np.asarray(lengths).astype(np.int64)
    trans = np.asarray(transitions, dtype=F32)

    transT = np.ascontiguousarray(trans.T)
    trans_flat = trans.reshape(-1)
    trans_gold = np.ascontiguousarray(trans_flat.reshape(512, 128).T)
    ones = np.ones((128, 1), dtype=F32)

    in_maps = []
    for c in range(N_CORES):
        sl = slice(c * BL, (c + 1) * BL)
        u = unary[sl]          # [16, 1024, 254]
        tg = tags[sl]          # [16, 1024]
        ln = lengths[sl]       # [16]

        u_pad = np.full((T, BL, N), NEG, dtype=BF)
        u_pad[:, :, :NT] = np.transpose(u, (1, 0, 2)).astype(BF)

        tmask = np.arange(T)[None, :] < ln[:, None]
        tg_m = np.where(tmask, tg, 300)
        O = (np.arange(N)[:, None, None] == tg_m.T[None, :, :]).astype(BF)

        cnt = np.zeros((N * N, BL), dtype=F32)
        prev = np.concatenate([np.full((BL, 1), NT, dtype=np.int64),
                               tg[:, :-1]], axis=1)
        flat = (tg * N + prev)  # [16, 1024]
        for b in range(BL):
            np.add.at(cnt[:, b], flat[b, :ln[b]], 1.0)
            last = tg[b, ln[b] - 1]
            cnt[(NT + 1) * N + last, b] += 1.0
        cnt_dev = np.ascontiguousarray(
            cnt.reshape(512, 128, BL).transpose(1, 0, 2).reshape(128, 512 * BL))

        p0 = np.zeros((N, BL), dtype=BF)
        p0[NT, :] = 1.0

        msel = np.zeros((NGRP, GRP * BL), dtype=F32)
        for b in range(BL):
            tsel = int(ln[b]) - 1
            msel[tsel // GRP, (tsel % GRP) * BL + b] = 1.0

        lenc = (ln.astype(F32) * CSTAR).reshape(1, BL)

        in_maps.append({
            "u_pad": np.ascontiguousarray(u_pad.reshape(T * BL, N)),
            "onehot": np.ascontiguousarray(O.reshape(N, T * BL)),
            "cnt": cnt_dev,
            "transT": transT,
            "trans_gold": trans_gold,
            "p0": p0,
            "msel": msel,
            "lenc": lenc,
            "ones": ones,
        })
    return in_maps


def kernel(unary, tags, lengths, transitions):
    if "nc" not in _compiled:
        _compiled["nc"] = _build_nc()
    nc = _compiled["nc"]
    in_maps = _host_prep(unary, tags, lengths, transitions)
    import os
    trace = bool(os.environ.get("CRF_TRACE"))
    res = run_bass_kernel_spmd(nc, in_maps, core_ids=list(range(N_CORES)),
                               trace=trace)
    if trace:
        _compiled["last_result"] = res
    out = np.concatenate([res.results[c]["out"].reshape(BL) for c in range(N_CORES)])
    return out.astype(F32)
